# revision 8
# baseline (speedup 1.0000x reference)
"""DynamicLinear (MoE routing) Trainium2 Bass kernel.

Math (per sample b):
    out[b] = sum_k attn[b,k] * (x[b] @ W[k].T + bias[k])
           = sum_k attn[b,k] * (x[b] @ W[k].T) + attn[b] @ bias

Sharding: 8 cores in a 2x4 grid over (batch, out_features).
Each core computes out[b_half, o_quarter] from x[b_half] (8 MiB bf16)
and W[:, o_quarter, :] (8 MiB bf16) -- no cross-core communication.

The host ships x and W pre-tiled and pre-cast to bf16 in the exact
SBUF layouts the kernel consumes, so the device needs no casts,
transposes, or gathers. Matmuls run bf16 x bf16 with fp32 PSUM
accumulation (the compute roofline; fp8 DoubleRow is only ~1.44x on
TRN2 and needs >=3 matmuls to stay under the 2e-2 error budget, a net
loss; bf16 PSUM accumulation that would allow N=1024 matmuls is
TRN3-only).

Front schedule (the fill is HBM-delivery-bound at ~179 GB/s per HWDGE
ring; PE continuity is what keeps the HAM clock gate at 8/8):
- W0 streams as granules of [2,2,4,4,4] ii-tiles into subtile slices
  of one SBUF tile; x0/x1 are chunk-loaded ([8,8] and [4,12]) the
  same way. Tile's subtile deps let each matmul start as soon as the
  granule/chunk it reads has landed, pulling the first real matmul
  ~3 us earlier than whole-tile gating would.
- The k=0 passes for tiles 0..2 are emitted in readiness order
  (interleaved by predicted granule/chunk arrival), with small dummy
  groups at the three predicted delivery gaps so the PE never idles
  long enough to re-throttle.
- A zero-matmul warmup bridges the PE from sequencer-ready (~7.7 us)
  to first-granule-ready (~10.5 us), so the HAM busy window starts
  accumulating immediately and the clock flips to 2.4 GHz early.
- Queue plan (two HWDGE rings, FIFO within each, engines round-robin
  between them): sync carries W0 granules then x4..x15 then W2/W3
  then the k=3 stores; scalar carries x0/x1 chunks, attn, x2, x3,
  then W1 and bias which land well before their ~70 us consumers.
- The bias madds are deferred to the k=1/k=2 combines: off both the
  fill window and the final-store tail.
- The last-processed tile (k=3, t=15) runs as 256/192/64-column
  passes with separate PSUM tiles, accs, and store queues: each
  chunk's combine+descriptor-gen hides under the next chunk's
  matmuls, and the tail after the last matmul is one 64-column
  combine + 32 KiB store before the fixed drain.
"""

import numpy as np

_B, _K, _IN, _OUT = 4096, 4, 2048, 2048
_GRID_B, _GRID_O = 2, 4
_BL = _B // _GRID_B      # 2048 batch rows per core
_OL = _OUT // _GRID_O    # 512 out cols per core
_NBT = _BL // 128        # 16 b tiles
_NIT = _IN // 128        # 16 contraction tiles

# --- tuning knobs ---
_NWARM = 16              # N=256 zero-matmul warmup count (ends at the
                         # measured first-chunk delivery time ~10.6 us,
                         # and is 3.4 us contiguous so the HAM clock-gate
                         # flip is guaranteed by ~14.3 us)
# k=0 fill emission: (pass, ii0, ii1, n_dummies_after, combine_after),
# need-sorted against the predicted granule/chunk arrival times
_FILL = [
    (0, 0, 1, 0, False),
    (0, 1, 2, 0, False),
    (1, 0, 2, 0, False),
    (0, 2, 3, 0, False),
    (1, 2, 4, 0, False),
    (0, 3, 4, 0, False),
    (0, 4, 6, 0, False),
    (0, 6, 8, 0, False),
    (1, 4, 8, 0, False),
    (2, 0, 4, 0, False),
    (0, 8, 12, 0, False),
    (1, 8, 12, 0, False),
    (2, 4, 8, 0, False),
    (0, 12, 16, 0, True),
    (1, 12, 16, 0, True),
    (2, 8, 12, 0, False),
    (2, 12, 16, 0, True),
]
_NFILL = 3               # tiles covered by the fill emission
_G0 = (1, 1, 1, 1, 2, 2, 4, 4)   # W0 granule ii sizes
_X0CH = (4, 12)          # x0 chunk ii sizes
_X1CH = (4, 12)          # x1 chunk ii sizes
_X2CH = (4, 12)          # x2 chunk ii sizes
_CHUNKS = (256, 192, 64)  # column chunks of the final (k=3, t=15) pass

_CACHE = {}
LAST_RESULTS = None


def _build_program():
    import concourse.bass as bass
    import concourse.tile as tile
    from concourse import bacc, mybir

    f32 = mybir.dt.float32
    MULT = mybir.AluOpType.mult
    ADD = mybir.AluOpType.add

    nc = bacc.Bacc("TRN2", target_bir_lowering=False, debug=False)
    bf16 = mybir.dt.bfloat16
    xT = nc.dram_tensor("xT", [_NBT, 128, _NIT, 128], bf16,
                        kind="ExternalInput").ap()
    # host-pretiled: attn[p, t, k] = softmax_attention[t*128 + p, k]
    attn = nc.dram_tensor("attn", [128, _NBT, _K], f32,
                          kind="ExternalInput").ap()
    wT = nc.dram_tensor("wT", [_K, 128, _NIT, _OL], bf16,
                        kind="ExternalInput").ap()
    # host-pre-replicated across partitions (an on-device broadcast
    # runs on the slow SWDGE path, ~52 us of shared DMA-engine time)
    bias = nc.dram_tensor("bias", [128, _K, _OL], f32,
                          kind="ExternalInput").ap()
    out = nc.dram_tensor("out", [_BL, _OL], f32, kind="ExternalOutput").ap()

    GH = 4            # ii-tiles per W granule, experts 1..3
    csum = [0]
    for c in _CHUNKS:
        csum.append(csum[-1] + c)
    assert csum[-1] == _OL

    with tile.TileContext(nc) as tc:
        with (
            tc.tile_pool(name="wt0", bufs=1) as wt0p,
            tc.tile_pool(name="wt", bufs=3 * (_NIT // GH)) as wtp,
            tc.tile_pool(name="xt", bufs=_NBT) as xtp,
            tc.tile_pool(name="singles", bufs=1) as singles,
            tc.tile_pool(name="acc", bufs=_NBT - 1) as accp,
            tc.tile_pool(name="acc15", bufs=1) as acc15p,
            tc.tile_pool(name="psum", bufs=5, space="PSUM") as psump,
            tc.tile_pool(name="psumh", bufs=1, space="PSUM") as psumhp,
        ):
            # --- PE warmup: zero matmuls bridging the PE from
            # sequencer-ready to first-granule-ready, keeping the HAM
            # busy window accumulating from ~7.7 us.
            warm = singles.tile([128, 256], bf16, tag="warm", name="warm")
            nc.vector.memset(warm, 0.0)
            ps_warm = psump.tile([128, 256], f32, tag="ps", name="ps_warm")
            for i in range(_NWARM):
                nc.tensor.matmul(
                    ps_warm, lhsT=warm[:, 0:128], rhs=warm,
                    start=(i == 0), stop=(i == _NWARM - 1),
                )

            # --- SBUF tiles; wt0/x0/x1 are filled by multiple chunked
            # DMAs into subtile slices (deps resolve per chunk) ---
            wt0_t = wt0p.tile([128, _NIT, _OL], bf16, tag="wt0full",
                              name="wt0")
            xts = {t: xtp.tile([128, _NIT, 128], bf16, tag="xt",
                               name=f"xt{t}")
                   for t in range(_NBT)}

            def load_g(i0, i1, q):
                q.dma_start(out=wt0_t[:, i0:i1, :], in_=wT[0, :, i0:i1])

            def load_xch(t, i0, i1, q):
                q.dma_start(out=xts[t][:, i0:i1, :], in_=xT[t, :, i0:i1])

            # sync ring: W0 granules, then x4..x15, then W2/W3
            h = 0
            for g in _G0:
                load_g(h, h + g, nc.sync)
                h += g
            # scalar ring: x0/x1 chunks, attn, x2, x3, x4, W1, bias
            load_xch(0, 0, _X0CH[0], nc.scalar)
            load_xch(1, 0, _X1CH[0], nc.scalar)
            load_xch(0, _X0CH[0], _NIT, nc.scalar)
            attn_sb = singles.tile([128, _NBT, _K], f32, tag="attn")
            nc.scalar.dma_start(out=attn_sb, in_=attn)
            load_xch(1, _X1CH[0], _NIT, nc.scalar)
            load_xch(2, 0, _X2CH[0], nc.scalar)
            load_xch(2, _X2CH[0], _NIT, nc.scalar)
            nc.scalar.dma_start(out=xts[3], in_=xT[3])
            # x4 rides scalar too: the extra sync-ring granule gens push
            # sync's x stream just past t=4's deadline otherwise
            nc.scalar.dma_start(out=xts[4], in_=xT[4])
            for t in range(5, _NBT):
                nc.sync.dma_start(out=xts[t], in_=xT[t])
            wt = {}

            def load_w(k, hh, q):
                t_ = wtp.tile([128, GH, _OL], bf16, tag="wt",
                              name=f"wt{k}_{hh}")
                q.dma_start(out=t_, in_=wT[k, :, hh * GH:(hh + 1) * GH])
                return t_

            for hh in range(_NIT // GH):
                wt[(1, hh)] = load_w(1, hh, nc.scalar)
            bias_rep = singles.tile([128, _K, _OL], f32, tag="bias")
            nc.scalar.dma_start(out=bias_rep, in_=bias)
            for k in (2, 3):
                for hh in range(_NIT // GH):
                    wt[(k, hh)] = load_w(k, hh, nc.sync)

            def w_slice(k, ii, c0=0, c1=_OL):
                if k == 0:
                    return wt0_t[:, ii, c0:c1]
                return wt[(k, ii // GH)][:, ii % GH, c0:c1]

            acc = [None] * _NBT        # full tiles for t < 15
            acc15 = [None] * len(_CHUNKS)   # column chunks for t = 15

            def combine(k, t, ps_ap, a_sc, c0, c1, which):
                # acc update for columns [c0:c1); which selects the acc
                at = acc[t] if t < _NBT - 1 else acc15[which]
                if k == 0:
                    # init: acc = a_0 * psum  (bias terms deferred)
                    nc.vector.tensor_scalar(
                        out=at, in0=ps_ap, scalar1=a_sc[:, 0:1],
                        scalar2=None, op0=MULT,
                    )
                else:
                    nc.vector.scalar_tensor_tensor(
                        out=at, in0=ps_ap, scalar=a_sc[:, k:k + 1],
                        in1=at, op0=MULT, op1=ADD,
                    )
                # bias madds folded into the k=1/k=2 combines (2 each):
                # off both the fill window and the store tail
                if k in (1, 2):
                    for kk in ((0, 1) if k == 1 else (2, 3)):
                        nc.vector.scalar_tensor_tensor(
                            out=at, in0=bias_rep[:, kk, c0:c1],
                            scalar=a_sc[:, kk:kk + 1], in1=at,
                            op0=MULT, op1=ADD,
                        )

            # --- fill: k=0 passes for tiles 0.._NFILL-1, emitted in
            # readiness order over the granule/chunk stream
            ps_f = [psump.tile([128, _OL], f32, tag="ps", name=f"ps0_{p}")
                    for p in range(_NFILL)]
            for (p, i0, i1, nd, comb) in _FILL:
                for ii in range(i0, i1):
                    nc.tensor.matmul(
                        ps_f[p], lhsT=xts[p][:, ii, :], rhs=w_slice(0, ii),
                        start=(ii == 0), stop=(ii == _NIT - 1),
                    )
                for j in range(nd):
                    nc.tensor.matmul(
                        ps_warm[:, 0:128], lhsT=warm[:, 0:128],
                        rhs=warm[:, 0:128],
                        start=(j == 0), stop=(j == nd - 1),
                    )
                if comb:
                    acc[p] = accp.tile([128, _OL], f32, tag="acc",
                                       name=f"acc{p}")
                    combine(0, p, ps_f[p], attn_sb[:, p, :], 0, _OL, 0)

            # rest of the k=0 sweep
            for t in range(_NFILL, _NBT):
                xt = xts[t]
                a_sc = attn_sb[:, t, :]
                ps = psump.tile([128, _OL], f32, tag="ps", name=f"ps0_{t}")
                for ii in range(_NIT):
                    nc.tensor.matmul(
                        ps, lhsT=xt[:, ii, :], rhs=w_slice(0, ii),
                        start=(ii == 0), stop=(ii == _NIT - 1),
                    )
                if t < _NBT - 1:
                    acc[t] = accp.tile([128, _OL], f32, tag="acc",
                                       name=f"acc{t}")
                    combine(0, t, ps, a_sc, 0, _OL, 0)
                else:
                    for w_, (c0, c1) in enumerate(zip(csum, csum[1:])):
                        acc15[w_] = acc15p.tile(
                            [128, c1 - c0], f32, tag=f"acc15_{w_}",
                            name=f"acc15_{w_}")
                        combine(0, t, ps[:, c0:c1], a_sc, c0, c1, w_)

            # k=1..3 sweeps; t=15's k=3 pass runs as column chunks so
            # each chunk's combine + store-descriptor-gen hides under
            # the next chunk's matmuls and the tail is one small
            # combine + 32 KiB store
            for k in (1, 2, 3):
                for t in range(_NBT):
                    xt = xts[t]
                    a_sc = attn_sb[:, t, :]
                    last_tile = (t == _NBT - 1)
                    if k == 3 and last_tile:
                        for w_, (c0, c1) in enumerate(zip(csum, csum[1:])):
                            ph = psumhp.tile([128, c1 - c0], f32,
                                             tag=f"psh{w_}",
                                             name=f"psh{w_}")
                            for ii in range(_NIT):
                                nc.tensor.matmul(
                                    ph, lhsT=xt[:, ii, :],
                                    rhs=w_slice(3, ii, c0, c1),
                                    start=(ii == 0),
                                    stop=(ii == _NIT - 1),
                                )
                            combine(3, t, ph, a_sc, c0, c1, w_)
                            q = nc.scalar if w_ == 1 else nc.sync
                            q.dma_start(
                                out=out[t * 128:(t + 1) * 128, c0:c1],
                                in_=acc15[w_],
                            )
                        continue
                    ps = psump.tile([128, _OL], f32, tag="ps",
                                    name=f"ps{k}_{t}")
                    for ii in range(_NIT):
                        nc.tensor.matmul(
                            ps, lhsT=xt[:, ii, :], rhs=w_slice(k, ii),
                            start=(ii == 0), stop=(ii == _NIT - 1),
                        )
                    if last_tile:
                        for w_, (c0, c1) in enumerate(zip(csum, csum[1:])):
                            combine(k, t, ps[:, c0:c1], a_sc, c0, c1, w_)
                    else:
                        combine(k, t, ps, a_sc, 0, _OL, 0)
                        if k == 3:
                            nc.sync.dma_start(
                                out=out[t * 128:(t + 1) * 128, :],
                                in_=acc[t],
                            )

    nc.compile()
    return nc


def _get_program():
    if "nc" not in _CACHE:
        _CACHE["nc"] = _build_program()
    return _CACHE["nc"]


def _ensure_axon_hooks_importable():
    """bass_utils' trace branch imports antenv.axon_hooks, which the
    trimmed agent image may lack; stub it (hook=None) so a stray
    BASS_TRACE=1 degrades to an untraced run instead of crashing."""
    import sys
    import types

    try:
        import antenv.axon_hooks  # noqa: F401
        return
    except ImportError:
        pass
    mod = types.ModuleType("antenv.axon_hooks")
    mod._hook = None
    mod.get_axon_ntff_profile_hook = lambda: mod._hook

    def _set(h):
        mod._hook = h

    mod.set_axon_ntff_profile_hook = _set
    sys.modules["antenv.axon_hooks"] = mod
    try:
        import antenv
        antenv.axon_hooks = mod
    except ImportError:
        pass


def kernel(**inputs):
    global LAST_RESULTS
    from concourse.bass_utils import run_bass_kernel_spmd

    _ensure_axon_hooks_importable()

    x = np.ascontiguousarray(inputs["x"], dtype=np.float32)
    attn = np.ascontiguousarray(inputs["softmax_attention"], dtype=np.float32)
    w = np.ascontiguousarray(inputs["weight"], dtype=np.float32)
    b = np.ascontiguousarray(inputs["bias"], dtype=np.float32)

    nc = _get_program()
    in_maps = []
    for c in range(8):
        gb, go = divmod(c, _GRID_O)
        x_sl = x[gb * _BL:(gb + 1) * _BL]
        w_sl = w[:, go * _OL:(go + 1) * _OL, :]
        # tile-contiguous device layouts (see _build_program):
        # xT[t, i_in, ii, b_in] = x[t*128 + b_in, ii*128 + i_in]
        # wT[k, i_in, ii, o]    = W[k, o, ii*128 + i_in]
        import ml_dtypes
        xT = np.ascontiguousarray(
            x_sl.T.reshape(_NIT, 128, _NBT, 128).transpose(2, 1, 0, 3)
        ).astype(ml_dtypes.bfloat16)
        wTa = np.ascontiguousarray(
            w_sl.transpose(0, 2, 1)
            .reshape(_K, _NIT, 128, _OL).transpose(0, 2, 1, 3)
        ).astype(ml_dtypes.bfloat16)
        # attnT[p, t, k] = attn[gb*BL + t*128 + p, k]
        attnT = np.ascontiguousarray(
            attn[gb * _BL:(gb + 1) * _BL]
            .reshape(_NBT, 128, _K).transpose(1, 0, 2)
        )
        in_maps.append({
            "xT": xT,
            "attn": attnT,
            "wT": wTa,
            "bias": np.ascontiguousarray(np.broadcast_to(
                b[None, :, go * _OL:(go + 1) * _OL], (128, _K, _OL))),
        })

    res = run_bass_kernel_spmd(nc, in_maps, list(range(8)))
    LAST_RESULTS = res

    full = np.empty((_B, _OUT), dtype=np.float32)
    for c in range(8):
        gb, go = divmod(c, _GRID_O)
        full[gb * _BL:(gb + 1) * _BL, go * _OL:(go + 1) * _OL] = \
            res.results[c]["out"]
    return full


# revision 9
# speedup vs baseline: 1.0011x; 1.0011x over previous
"""DynamicLinear (MoE routing) Trainium2 Bass kernel.

Math (per sample b):
    out[b] = sum_k attn[b,k] * (x[b] @ W[k].T + bias[k])
           = sum_k attn[b,k] * (x[b] @ W[k].T) + attn[b] @ bias

Sharding: 8 cores in a 2x4 grid over (batch, out_features).
Each core computes out[b_half, o_quarter] from x[b_half] (8 MiB bf16)
and W[:, o_quarter, :] (8 MiB bf16) -- no cross-core communication.

The host ships x and W pre-tiled and pre-cast to bf16 in the exact
SBUF layouts the kernel consumes, so the device needs no casts,
transposes, or gathers. Matmuls run bf16 x bf16 with fp32 PSUM
accumulation (the compute roofline; fp8 DoubleRow is only ~1.44x on
TRN2 and needs >=3 matmuls to stay under the 2e-2 error budget, a net
loss; bf16 PSUM accumulation that would allow N=1024 matmuls is
TRN3-only).

Front schedule (the fill is HBM-delivery-bound at ~179 GB/s per HWDGE
ring; PE continuity is what keeps the HAM clock gate at 8/8):
- W0 streams as granules of [2,2,4,4,4] ii-tiles into subtile slices
  of one SBUF tile; x0/x1 are chunk-loaded ([8,8] and [4,12]) the
  same way. Tile's subtile deps let each matmul start as soon as the
  granule/chunk it reads has landed, pulling the first real matmul
  ~3 us earlier than whole-tile gating would.
- The k=0 passes for tiles 0..2 are emitted in readiness order
  (interleaved by predicted granule/chunk arrival), with small dummy
  groups at the three predicted delivery gaps so the PE never idles
  long enough to re-throttle.
- A zero-matmul warmup bridges the PE from sequencer-ready (~7.7 us)
  to first-granule-ready (~10.5 us), so the HAM busy window starts
  accumulating immediately and the clock flips to 2.4 GHz early.
- Queue plan (two HWDGE rings, FIFO within each, engines round-robin
  between them): sync carries W0 granules then x4..x15 then W2/W3
  then the k=3 stores; scalar carries x0/x1 chunks, attn, x2, x3,
  then W1 and bias which land well before their ~70 us consumers.
- The bias madds are deferred to the k=1/k=2 combines: off both the
  fill window and the final-store tail.
- The last-processed tile (k=3, t=15) runs as 256/192/64-column
  passes with separate PSUM tiles, accs, and store queues: each
  chunk's combine+descriptor-gen hides under the next chunk's
  matmuls, and the tail after the last matmul is one 64-column
  combine + 32 KiB store before the fixed drain.
"""

import numpy as np

_B, _K, _IN, _OUT = 4096, 4, 2048, 2048
_GRID_B, _GRID_O = 2, 4
_BL = _B // _GRID_B      # 2048 batch rows per core
_OL = _OUT // _GRID_O    # 512 out cols per core
_NBT = _BL // 128        # 16 b tiles
_NIT = _IN // 128        # 16 contraction tiles

# --- tuning knobs ---
_NWARM = 16              # N=256 zero-matmul warmup count (ends at the
                         # measured first-chunk delivery time ~10.6 us,
                         # and is 3.4 us contiguous so the HAM clock-gate
                         # flip is guaranteed by ~14.3 us)
# k=0 fill emission: (pass, ii0, ii1, n_dummies_after, combine_after),
# need-sorted against the predicted granule/chunk arrival times; the
# dummy groups pad every predicted delivery stall so the PE never goes
# idle (scattered sub-us gaps re-throttle the HAM clock gate)
_FILL = [
    (0, 0, 1, 2, False),
    (1, 0, 2, 0, False),
    (0, 1, 2, 2, False),
    (0, 2, 4, 2, False),
    (1, 2, 4, 18, False),
    (2, 0, 4, 0, False),
    (0, 4, 8, 0, False),
    (1, 4, 8, 0, False),
    (2, 4, 8, 0, False),
    (0, 8, 12, 0, False),
    (1, 8, 12, 8, False),
    (0, 12, 16, 0, True),
    (1, 12, 16, 0, True),
    (2, 8, 12, 0, False),
    (2, 12, 16, 0, True),
]
_NFILL = 3               # tiles covered by the fill emission
_G0 = (1, 1, 2, 4, 4, 4)   # W0 granule ii sizes
_X0CH = (4, 12)          # x0 chunk ii sizes
_X1CH = (4, 12)          # x1 chunk ii sizes
_X2CH = (4, 12)          # x2 chunk ii sizes
_CHUNKS = (256, 192, 64)  # column chunks of the final (k=3, t=15) pass

_CACHE = {}
LAST_RESULTS = None


def _build_program():
    import concourse.bass as bass
    import concourse.tile as tile
    from concourse import bacc, mybir

    f32 = mybir.dt.float32
    MULT = mybir.AluOpType.mult
    ADD = mybir.AluOpType.add

    nc = bacc.Bacc("TRN2", target_bir_lowering=False, debug=False)
    bf16 = mybir.dt.bfloat16
    xT = nc.dram_tensor("xT", [_NBT, 128, _NIT, 128], bf16,
                        kind="ExternalInput").ap()
    # host-pretiled: attn[p, t, k] = softmax_attention[t*128 + p, k]
    attn = nc.dram_tensor("attn", [128, _NBT, _K], f32,
                          kind="ExternalInput").ap()
    wT = nc.dram_tensor("wT", [_K, 128, _NIT, _OL], bf16,
                        kind="ExternalInput").ap()
    # host-pre-replicated across partitions (an on-device broadcast
    # runs on the slow SWDGE path, ~52 us of shared DMA-engine time)
    bias = nc.dram_tensor("bias", [128, _K, _OL], f32,
                          kind="ExternalInput").ap()
    out = nc.dram_tensor("out", [_BL, _OL], f32, kind="ExternalOutput").ap()

    GH = 4            # ii-tiles per W granule, experts 1..3
    csum = [0]
    for c in _CHUNKS:
        csum.append(csum[-1] + c)
    assert csum[-1] == _OL

    with tile.TileContext(nc) as tc:
        with (
            tc.tile_pool(name="wt0", bufs=1) as wt0p,
            tc.tile_pool(name="wt", bufs=3 * (_NIT // GH)) as wtp,
            tc.tile_pool(name="xt", bufs=_NBT) as xtp,
            tc.tile_pool(name="singles", bufs=1) as singles,
            tc.tile_pool(name="acc", bufs=_NBT - 1) as accp,
            tc.tile_pool(name="acc15", bufs=1) as acc15p,
            tc.tile_pool(name="psum", bufs=5, space="PSUM") as psump,
            tc.tile_pool(name="psumh", bufs=1, space="PSUM") as psumhp,
        ):
            # --- PE warmup: zero matmuls bridging the PE from
            # sequencer-ready to first-granule-ready, keeping the HAM
            # busy window accumulating from ~7.7 us.
            warm = singles.tile([128, 256], bf16, tag="warm", name="warm")
            nc.vector.memset(warm, 0.0)
            ps_warm = psump.tile([128, 256], f32, tag="ps", name="ps_warm")
            for i in range(_NWARM):
                nc.tensor.matmul(
                    ps_warm, lhsT=warm[:, 0:128], rhs=warm,
                    start=(i == 0), stop=(i == _NWARM - 1),
                )

            # --- SBUF tiles; wt0/x0/x1 are filled by multiple chunked
            # DMAs into subtile slices (deps resolve per chunk) ---
            wt0_t = wt0p.tile([128, _NIT, _OL], bf16, tag="wt0full",
                              name="wt0")
            xts = {t: xtp.tile([128, _NIT, 128], bf16, tag="xt",
                               name=f"xt{t}")
                   for t in range(_NBT)}

            def load_g(i0, i1, q):
                q.dma_start(out=wt0_t[:, i0:i1, :], in_=wT[0, :, i0:i1])

            def load_xch(t, i0, i1, q):
                q.dma_start(out=xts[t][:, i0:i1, :], in_=xT[t, :, i0:i1])

            # sync ring: W0 granules, then x4..x15, then W2/W3
            h = 0
            for g in _G0:
                load_g(h, h + g, nc.sync)
                h += g
            # scalar ring: x0/x1 chunks, attn, x2, x3, x4, W1, bias
            load_xch(0, 0, _X0CH[0], nc.scalar)
            load_xch(1, 0, _X1CH[0], nc.scalar)
            load_xch(0, _X0CH[0], _NIT, nc.scalar)
            attn_sb = singles.tile([128, _NBT, _K], f32, tag="attn")
            nc.scalar.dma_start(out=attn_sb, in_=attn)
            load_xch(1, _X1CH[0], _NIT, nc.scalar)
            load_xch(2, 0, _X2CH[0], nc.scalar)
            load_xch(2, _X2CH[0], _NIT, nc.scalar)
            nc.scalar.dma_start(out=xts[3], in_=xT[3])
            # x4 rides scalar too: the extra sync-ring granule gens push
            # sync's x stream just past t=4's deadline otherwise
            nc.scalar.dma_start(out=xts[4], in_=xT[4])
            for t in range(5, _NBT):
                nc.sync.dma_start(out=xts[t], in_=xT[t])
            wt = {}

            def load_w(k, hh, q):
                t_ = wtp.tile([128, GH, _OL], bf16, tag="wt",
                              name=f"wt{k}_{hh}")
                q.dma_start(out=t_, in_=wT[k, :, hh * GH:(hh + 1) * GH])
                return t_

            for hh in range(_NIT // GH):
                wt[(1, hh)] = load_w(1, hh, nc.scalar)
            bias_rep = singles.tile([128, _K, _OL], f32, tag="bias")
            nc.scalar.dma_start(out=bias_rep, in_=bias)
            for k in (2, 3):
                for hh in range(_NIT // GH):
                    wt[(k, hh)] = load_w(k, hh, nc.sync)

            def w_slice(k, ii, c0=0, c1=_OL):
                if k == 0:
                    return wt0_t[:, ii, c0:c1]
                return wt[(k, ii // GH)][:, ii % GH, c0:c1]

            acc = [None] * _NBT        # full tiles for t < 15
            acc15 = [None] * len(_CHUNKS)   # column chunks for t = 15

            def combine(k, t, ps_ap, a_sc, c0, c1, which):
                # acc update for columns [c0:c1); which selects the acc
                at = acc[t] if t < _NBT - 1 else acc15[which]
                if k == 0:
                    # init: acc = a_0 * psum  (bias terms deferred)
                    nc.vector.tensor_scalar(
                        out=at, in0=ps_ap, scalar1=a_sc[:, 0:1],
                        scalar2=None, op0=MULT,
                    )
                else:
                    nc.vector.scalar_tensor_tensor(
                        out=at, in0=ps_ap, scalar=a_sc[:, k:k + 1],
                        in1=at, op0=MULT, op1=ADD,
                    )
                # bias madds folded into the k=1/k=2 combines (2 each):
                # off both the fill window and the store tail
                if k in (1, 2):
                    for kk in ((0, 1) if k == 1 else (2, 3)):
                        nc.vector.scalar_tensor_tensor(
                            out=at, in0=bias_rep[:, kk, c0:c1],
                            scalar=a_sc[:, kk:kk + 1], in1=at,
                            op0=MULT, op1=ADD,
                        )

            # --- fill: k=0 passes for tiles 0.._NFILL-1, emitted in
            # readiness order over the granule/chunk stream
            ps_f = [psump.tile([128, _OL], f32, tag="ps", name=f"ps0_{p}")
                    for p in range(_NFILL)]
            for (p, i0, i1, nd, comb) in _FILL:
                for ii in range(i0, i1):
                    nc.tensor.matmul(
                        ps_f[p], lhsT=xts[p][:, ii, :], rhs=w_slice(0, ii),
                        start=(ii == 0), stop=(ii == _NIT - 1),
                    )
                for j in range(nd):
                    nc.tensor.matmul(
                        ps_warm[:, 0:128], lhsT=warm[:, 0:128],
                        rhs=warm[:, 0:128],
                        start=(j == 0), stop=(j == nd - 1),
                    )
                if comb:
                    acc[p] = accp.tile([128, _OL], f32, tag="acc",
                                       name=f"acc{p}")
                    combine(0, p, ps_f[p], attn_sb[:, p, :], 0, _OL, 0)

            # rest of the k=0 sweep
            for t in range(_NFILL, _NBT):
                xt = xts[t]
                a_sc = attn_sb[:, t, :]
                ps = psump.tile([128, _OL], f32, tag="ps", name=f"ps0_{t}")
                for ii in range(_NIT):
                    nc.tensor.matmul(
                        ps, lhsT=xt[:, ii, :], rhs=w_slice(0, ii),
                        start=(ii == 0), stop=(ii == _NIT - 1),
                    )
                if t < _NBT - 1:
                    acc[t] = accp.tile([128, _OL], f32, tag="acc",
                                       name=f"acc{t}")
                    combine(0, t, ps, a_sc, 0, _OL, 0)
                else:
                    for w_, (c0, c1) in enumerate(zip(csum, csum[1:])):
                        acc15[w_] = acc15p.tile(
                            [128, c1 - c0], f32, tag=f"acc15_{w_}",
                            name=f"acc15_{w_}")
                        combine(0, t, ps[:, c0:c1], a_sc, c0, c1, w_)

            # k=1..3 sweeps; t=15's k=3 pass runs as column chunks so
            # each chunk's combine + store-descriptor-gen hides under
            # the next chunk's matmuls and the tail is one small
            # combine + 32 KiB store
            for k in (1, 2, 3):
                for t in range(_NBT):
                    xt = xts[t]
                    a_sc = attn_sb[:, t, :]
                    last_tile = (t == _NBT - 1)
                    if k == 3 and last_tile:
                        for w_, (c0, c1) in enumerate(zip(csum, csum[1:])):
                            ph = psumhp.tile([128, c1 - c0], f32,
                                             tag=f"psh{w_}",
                                             name=f"psh{w_}")
                            for ii in range(_NIT):
                                nc.tensor.matmul(
                                    ph, lhsT=xt[:, ii, :],
                                    rhs=w_slice(3, ii, c0, c1),
                                    start=(ii == 0),
                                    stop=(ii == _NIT - 1),
                                )
                            combine(3, t, ph, a_sc, c0, c1, w_)
                            q = nc.scalar if w_ == 1 else nc.sync
                            q.dma_start(
                                out=out[t * 128:(t + 1) * 128, c0:c1],
                                in_=acc15[w_],
                            )
                        continue
                    ps = psump.tile([128, _OL], f32, tag="ps",
                                    name=f"ps{k}_{t}")
                    for ii in range(_NIT):
                        nc.tensor.matmul(
                            ps, lhsT=xt[:, ii, :], rhs=w_slice(k, ii),
                            start=(ii == 0), stop=(ii == _NIT - 1),
                        )
                    if last_tile:
                        for w_, (c0, c1) in enumerate(zip(csum, csum[1:])):
                            combine(k, t, ps[:, c0:c1], a_sc, c0, c1, w_)
                    else:
                        combine(k, t, ps, a_sc, 0, _OL, 0)
                        if k == 3:
                            nc.sync.dma_start(
                                out=out[t * 128:(t + 1) * 128, :],
                                in_=acc[t],
                            )

    nc.compile()
    return nc


def _get_program():
    if "nc" not in _CACHE:
        _CACHE["nc"] = _build_program()
    return _CACHE["nc"]


def _ensure_axon_hooks_importable():
    """bass_utils' trace branch imports antenv.axon_hooks, which the
    trimmed agent image may lack; stub it (hook=None) so a stray
    BASS_TRACE=1 degrades to an untraced run instead of crashing."""
    import sys
    import types

    try:
        import antenv.axon_hooks  # noqa: F401
        return
    except ImportError:
        pass
    mod = types.ModuleType("antenv.axon_hooks")
    mod._hook = None
    mod.get_axon_ntff_profile_hook = lambda: mod._hook

    def _set(h):
        mod._hook = h

    mod.set_axon_ntff_profile_hook = _set
    sys.modules["antenv.axon_hooks"] = mod
    try:
        import antenv
        antenv.axon_hooks = mod
    except ImportError:
        pass


def kernel(**inputs):
    global LAST_RESULTS
    from concourse.bass_utils import run_bass_kernel_spmd

    _ensure_axon_hooks_importable()

    x = np.ascontiguousarray(inputs["x"], dtype=np.float32)
    attn = np.ascontiguousarray(inputs["softmax_attention"], dtype=np.float32)
    w = np.ascontiguousarray(inputs["weight"], dtype=np.float32)
    b = np.ascontiguousarray(inputs["bias"], dtype=np.float32)

    nc = _get_program()
    in_maps = []
    for c in range(8):
        gb, go = divmod(c, _GRID_O)
        x_sl = x[gb * _BL:(gb + 1) * _BL]
        w_sl = w[:, go * _OL:(go + 1) * _OL, :]
        # tile-contiguous device layouts (see _build_program):
        # xT[t, i_in, ii, b_in] = x[t*128 + b_in, ii*128 + i_in]
        # wT[k, i_in, ii, o]    = W[k, o, ii*128 + i_in]
        import ml_dtypes
        xT = np.ascontiguousarray(
            x_sl.T.reshape(_NIT, 128, _NBT, 128).transpose(2, 1, 0, 3)
        ).astype(ml_dtypes.bfloat16)
        wTa = np.ascontiguousarray(
            w_sl.transpose(0, 2, 1)
            .reshape(_K, _NIT, 128, _OL).transpose(0, 2, 1, 3)
        ).astype(ml_dtypes.bfloat16)
        # attnT[p, t, k] = attn[gb*BL + t*128 + p, k]
        attnT = np.ascontiguousarray(
            attn[gb * _BL:(gb + 1) * _BL]
            .reshape(_NBT, 128, _K).transpose(1, 0, 2)
        )
        in_maps.append({
            "xT": xT,
            "attn": attnT,
            "wT": wTa,
            "bias": np.ascontiguousarray(np.broadcast_to(
                b[None, :, go * _OL:(go + 1) * _OL], (128, _K, _OL))),
        })

    res = run_bass_kernel_spmd(nc, in_maps, list(range(8)))
    LAST_RESULTS = res

    full = np.empty((_B, _OUT), dtype=np.float32)
    for c in range(8):
        gb, go = divmod(c, _GRID_O)
        full[gb * _BL:(gb + 1) * _BL, go * _OL:(go + 1) * _OL] = \
            res.results[c]["out"]
    return full
